# revision 26
# baseline (speedup 1.0000x reference)
"""Trainium2 Bass kernel for nn_Encoder_80616536146562 (graph-LSTM encoder).

Reference computation (B=4, T=12, N=4096, F=16):
  per step t:
    gx = relu(adj @ (x_t @ Wx) + bx); gh = relu(adj @ (h @ Wh) + bh)
    LSTM gates -> c, h2
    sh = relu(adj @ (h2 @ Wsh) + bsh); sm = relu(adj @ (m @ Wsm) + bsm)
    memory gates -> m, h = m * o2
  outputs: hidden_states [B,T,N,F], last_h, last_c, last_m

Strategy: 8-way row-shard of adj (512 rows/core) kept resident in SBUF as
fp16 (scaled by 4096 to avoid fp16 subnormals; un-scaled on PSUM eviction).
Associativity: compute Y = adj @ z first (F cols per state), then tiny
Y @ W_blockdiag matmuls (batch folded block-diagonally, gate-major columns,
bias folded in via an appended ones-row on lhsT). Node states h/m/h2 are
all-gathered in fp16 across the 8 cores twice per step via collective_compute
through DRAM bounce buffers. The x-side conv for step t+1 (adj @ x_{t+1} and
its gate pre-activation) is computed inside step t's collective windows.
"""
import sys

if "/opt/trn_rl_repo" not in sys.path:
    sys.path.insert(0, "/opt/trn_rl_repo")

import numpy as np

B, T, N, F = 4, 12, 4096, 16
NCORES = 8
RPC = N // NCORES          # rows per core = 512
NM = RPC // 128            # m-tiles per core = 4
NK = N // 128              # k-tiles = 32
G1 = 4 * F                 # 64  (f|i|c|o gates)
G2 = 3 * F                 # 48  (i|g|o gates)
BF = B * F                 # 64
PBG1 = B * G1              # 256 per-m gate block
NG2 = B * G2               # 192
SCALE = 4096.0

_cache = {}


def _build_nc():
    import bass_rust as _bass_rust
    import concourse.bass as bass
    import concourse.mybir as mybir
    import concourse.tile as tile

    f32, f16 = mybir.dt.float32, mybir.dt.float16
    AF = mybir.ActivationFunctionType
    OP = mybir.AluOpType

    nc = bass.Bass(trn_type="TRN2", num_devices=NCORES)

    # ---- per-core inputs ----
    adjT = nc.dram_tensor("adjT", [N, RPC], f32, kind="ExternalInput")
    xr = nc.dram_tensor("xr", [T, N, BF], f32, kind="ExternalInput")
    # block-diagonal gate-major weights with bias as final (65th) row
    wx = nc.dram_tensor("wx", [BF + 1, PBG1], f32, kind="ExternalInput")
    wh = nc.dram_tensor("wh", [BF + 1, PBG1], f32, kind="ExternalInput")
    wsh = nc.dram_tensor("wsh", [BF + 1, NG2], f32, kind="ExternalInput")
    wsm = nc.dram_tensor("wsm", [BF + 1, NG2], f32, kind="ExternalInput")
    ident = nc.dram_tensor("ident", [128, 128], f32, kind="ExternalInput")
    # ---- per-core outputs ----
    hs = nc.dram_tensor("hs", [T, NM, B, 128, F], f32, kind="ExternalOutput")
    lc = nc.dram_tensor("lc", [NM, B, 128, F], f32, kind="ExternalOutput")
    lm = nc.dram_tensor("lm", [NM, B, 128, F], f32, kind="ExternalOutput")

    with tile.TileContext(nc) as tc:
        with tc.tile_pool(name="const", bufs=1) as constp, \
             tc.tile_pool(name="stage", bufs=2) as stagep, \
             tc.tile_pool(name="state", bufs=1) as statep, \
             tc.tile_pool(name="dram", bufs=2, space="DRAM") as dramp, \
             tc.tile_pool(name="work", bufs=2) as workp:

            # ===== constants / weights =====
            adj16 = [constp.tile([128, RPC], f16, name=f"adj16_{k}")
                     for k in range(NK)]
            for k in range(NK):
                a32 = stagep.tile([128, RPC], f32, tag="a32", bufs=6)
                nc.sync.dma_start(a32[:], adjT[k * 128:(k + 1) * 128, :])
                nc.vector.tensor_scalar_mul(adj16[k][:], a32[:], SCALE)

            id16 = constp.tile([128, 128], f16)
            i32 = stagep.tile([128, 128], f32, tag="i32")
            nc.sync.dma_start(i32[:], ident[:])
            nc.vector.tensor_copy(id16[:], i32[:])

            w16 = {}
            for name, t_, gw in (("wx", wx, PBG1), ("wh", wh, PBG1),
                                 ("wsh", wsh, NG2), ("wsm", wsm, NG2)):
                ww = stagep.tile([BF + 1, gw], f32, tag="w32", name=f"w32_{name}")
                nc.sync.dma_start(ww[:], t_[:])
                w1 = constp.tile([BF + 1, gw], f16, name=f"w16_{name}")
                nc.vector.tensor_copy(w1[:], ww[:])
                w16[name] = w1

            # ===== recurrent state =====
            # hm16[k] cols: [h (64) | m (64)]; x16t[k]: x_{t+1}
            hm16 = statep.tile([128, NK, 2 * BF], f16)
            x16t = statep.tile([128, NK, BF], f16)
            h2g = statep.tile([128, NK * BF], f16)        # gathered h2
            c32 = statep.tile([128, NM * BF], f32)
            m32 = statep.tile([128, NM * BF], f32)
            nc.gpsimd.memset(hm16[:], 1.0)
            nc.gpsimd.memset(c32[:], 1.0)
            nc.gpsimd.memset(m32[:], 1.0)

            with tc.tile_pool(name="py1", bufs=1, space="PSUM") as py1, \
                 tc.tile_pool(name="pt1", bufs=2, space="PSUM") as pt1, \
                 tc.tile_pool(name="pg", bufs=2, space="PSUM") as pgp:

                # ---- x-phase pieces ----
                def x_load(t):
                    """DMA + cast x_t into the x tile (for next stage 1)."""
                    xt32 = workp.tile([128, NK, BF], f32, tag="xt32")
                    for h in range(2):
                        nc.scalar.dma_start(
                            xt32[:, h * NK // 2:(h + 1) * NK // 2, :],
                            xr[t].rearrange("(k p) f -> p k f", p=128)
                            [:, h * NK // 2:(h + 1) * NK // 2, :])
                    nc.vector.tensor_copy(x16t[:], xt32[:])

                def x_phase_b(ax16):
                    """Transpose Ax, append ones row, rgx = relu(Ax@Wx + bx).
                    ax16: [128, NM, BF] fp16 view of un-scaled adj@x."""
                    axt_ps = pt1.tile([BF, NM * 128], f16, tag="sm1", name="axtps")
                    for m in range(NM):
                        nc.tensor.transpose(
                            axt_ps[:, m * 128:(m + 1) * 128], ax16[:, m, :],
                            id16[:])
                    axt = workp.tile([BF + 1, NM * 128], f16, tag="axt")
                    nc.scalar.copy(axt[0:BF, :], axt_ps[:])
                    nc.gpsimd.memset(axt[BF:BF + 1, :], 1.0)
                    gxa = pgp.tile([128, NM, PBG1], f32, tag="g", name="gxa")
                    for m in range(NM):
                        nc.tensor.matmul(
                            gxa[:, m, :], axt[:, m * 128:(m + 1) * 128],
                            w16["wx"][:], start=True, stop=True)
                    rgx = workp.tile([128, NM * PBG1], f32, tag="rgx")
                    nc.scalar.activation(
                        rgx[:], gxa[:].rearrange("p m g -> p (m g)"), AF.Relu)
                    return rgx

                # prologue: Ax for t=0 via a standalone matmul pass
                x_load(0)   # x_0 into the x columns
                axp0 = pt1.tile([128, NM, BF], f32, tag="sm1", name="axp0")
                for m in range(NM):
                    for k in range(NK):
                        nc.tensor.matmul(
                            axp0[:, m, :],
                            adj16[k][:, m * 128:(m + 1) * 128],
                            x16t[:, k, :],
                            start=(k == 0), stop=(k == NK - 1))
                ax0_16 = workp.tile([128, NM, BF], f16, tag="ax16")
                nc.scalar.mul(ax0_16[:], axp0[:], 1.0 / SCALE)
                rgx_cur = x_phase_b(ax0_16)
                x_load(1)   # x_1 into the x columns for step 0's stage 1

                for t in range(T):
                    # ---- stage 1: Y1X = adj @ [h|m|x_{t+1}] ----
                    # x-part first: independent of the h|m gather, so the PE
                    # processes it during the AG_B wait
                    y1 = py1.tile([128, NM, 4 * BF], f32, tag="y1")
                    for m in range(NM):
                        for k in range(NK):
                            nc.tensor.matmul(
                                y1[:, m, 2 * BF:3 * BF],
                                adj16[k][:, m * 128:(m + 1) * 128],
                                x16t[:, k, :],
                                start=(k == 0), stop=(k == NK - 1))
                    for m in range(NM):
                        for k in range(NK):
                            nc.tensor.matmul(
                                y1[:, m, 0:2 * BF],
                                adj16[k][:, m * 128:(m + 1) * 128],
                                hm16[:, k, :],
                                start=(k == 0), stop=(k == NK - 1))
                    y1s = workp.tile([128, NM, 3 * BF], f16, tag="y1s")
                    nc.scalar.mul(y1s[:], y1[:, :, 0:3 * BF], 1.0 / SCALE)
                    t1h_ps = pt1.tile([BF, NM * 128], f16, tag="sm1")
                    t1m_ps = pt1.tile([BF, NM * 128], f16, tag="sm1", name="t1mps")
                    for m in range(NM):
                        nc.tensor.transpose(
                            t1h_ps[:, m * 128:(m + 1) * 128],
                            y1s[:, m, 0:BF], id16[:])
                        nc.tensor.transpose(
                            t1m_ps[:, m * 128:(m + 1) * 128],
                            y1s[:, m, BF:2 * BF], id16[:])
                    t1h = workp.tile([BF + 1, NM * 128], f16, tag="t1h")
                    nc.scalar.copy(t1h[0:BF, :], t1h_ps[:])
                    nc.gpsimd.memset(t1h[BF:BF + 1, :], 1.0)
                    t1m = workp.tile([BF + 1, NM * 128], f16, tag="t1m")
                    nc.vector.tensor_copy(t1m[0:BF, :], t1m_ps[:])
                    nc.gpsimd.memset(t1m[BF:BF + 1, :], 1.0)

                    gh = pgp.tile([128, NM, PBG1], f32, tag="g")
                    for m in range(NM):
                        nc.tensor.matmul(
                            gh[:, m, :], t1h[:, m * 128:(m + 1) * 128],
                            w16["wh"][:], start=True, stop=True)
                    # s = rgx + relu(gh + bh)   (bias already inside gh)
                    s = workp.tile([128, NM, PBG1], f32, tag="s", bufs=1)
                    nc.vector.scalar_tensor_tensor(
                        s[:].rearrange("p m g -> p (m g)"),
                        gh[:].rearrange("p m g -> p (m g)"), 0.0,
                        rgx_cur[:], OP.max, OP.add)
                    # gate-major: per m [f | i | c | o], (b,f) inner
                    nc.scalar.activation(s[:, :, 0:2 * BF], s[:, :, 0:2 * BF],
                                         AF.Sigmoid)
                    nc.scalar.activation(s[:, :, 2 * BF:3 * BF],
                                         s[:, :, 2 * BF:3 * BF], AF.Tanh)
                    nc.scalar.activation(s[:, :, 3 * BF:4 * BF],
                                         s[:, :, 3 * BF:4 * BF], AF.Sigmoid)
                    cv = c32[:].rearrange("p (m c) -> p m c", m=NM)
                    tmp1 = workp.tile([128, NM, BF], f32, tag="tmp1")
                    tmp2 = workp.tile([128, NM, BF], f32, tag="tmp2")
                    nc.vector.tensor_tensor(tmp1[:], s[:, :, 0:BF], cv, OP.mult)
                    nc.vector.tensor_tensor(tmp2[:], s[:, :, BF:2 * BF],
                                            s[:, :, 2 * BF:3 * BF], OP.mult)
                    nc.vector.tensor_tensor(cv, tmp1[:], tmp2[:], OP.add)
                    th = workp.tile([128, NM * BF], f32, tag="th")
                    nc.scalar.activation(th[:], c32[:], AF.Tanh)
                    h2_16 = workp.tile([128, NM, BF], f16, tag="h2_16")
                    nc.vector.tensor_tensor(
                        h2_16[:], s[:, :, 3 * BF:4 * BF],
                        th[:].rearrange("p (m c) -> p m c", m=NM), OP.mult)
                    if t + 2 < T:
                        x_load(t + 2)

                    # ---------- all-gather h2 ----------
                    agA_in = dramp.tile([RPC, BF], f16, tag="agA_in")
                    nc.sync.dma_start(
                        agA_in.rearrange("(m p) f -> p m f", p=128), h2_16[:])
                    agA_out = dramp.tile([N, BF], f16, tag="agA_out",
                                         addr_space="Shared")
                    nc.gpsimd.collective_compute(
                        "AllGather", OP.bypass,
                        replica_groups=[list(range(NCORES))],
                        ins=[agA_in[:]], outs=[agA_out[:]])

                    # window work: sm matmul (needs only t1m) + x conv of t+1
                    sm = pgp.tile([128, NM, PBG1], f32, tag="g", name="sm")
                    for m in range(NM):
                        nc.tensor.matmul(
                            sm[:, m, 0:NG2], t1m[:, m * 128:(m + 1) * 128],
                            w16["wsm"][:], start=True, stop=True)
                    rsm = workp.tile([128, NM, PBG1], f32, tag="rsm", bufs=1)
                    nc.scalar.activation(
                        rsm[:, :, 0:NG2], sm[:, :, 0:NG2], AF.Relu)
                    if t < T - 1:
                        rgx_next = x_phase_b(y1s[:, :, 2 * BF:3 * BF])

                    for q in range(4):
                        nc.sync.dma_start(
                            h2g[:, q * NK // 4 * BF:(q + 1) * NK // 4 * BF]
                            .rearrange("p (k f) -> p k f", k=NK // 4),
                            agA_out.rearrange("(k p) f -> p k f", p=128)
                            [:, q * NK // 4:(q + 1) * NK // 4, :])

                    # ---------- stage 2: Y2 = adj @ h2 ----------
                    y2 = pt1.tile([128, NM, BF], f32, tag="sm1", name="y2")
                    for m in range(NM):
                        for k in range(NK):
                            nc.tensor.matmul(
                                y2[:, m, :],
                                adj16[k][:, m * 128:(m + 1) * 128],
                                h2g[:, k * BF:(k + 1) * BF],
                                start=(k == 0), stop=(k == NK - 1))
                    y2s = workp.tile([128, NM, BF], f16, tag="y2s")
                    nc.scalar.mul(y2s[:], y2[:], 1.0 / SCALE)
                    t2_ps = pt1.tile([BF, NM * 128], f16, tag="sm1", name="t2ps")
                    for m in range(NM):
                        nc.tensor.transpose(
                            t2_ps[:, m * 128:(m + 1) * 128], y2s[:, m, :], id16[:])
                    t2 = workp.tile([BF + 1, NM * 128], f16, tag="t2")
                    nc.scalar.copy(t2[0:BF, :], t2_ps[:])
                    nc.gpsimd.memset(t2[BF:BF + 1, :], 1.0)

                    sh = pgp.tile([128, NM, PBG1], f32, tag="g", name="sh")
                    for m in range(NM):
                        nc.tensor.matmul(
                            sh[:, m, 0:NG2], t2[:, m * 128:(m + 1) * 128],
                            w16["wsh"][:], start=True, stop=True)
                    # s2 = sigmoid(relu(sh+bsh) + relu(sm+bsm)); gate-major i|g|o
                    s2 = workp.tile([128, NM, PBG1], f32, tag="s2", bufs=1)
                    nc.vector.scalar_tensor_tensor(
                        s2[:, :, 0:NG2], sh[:, :, 0:NG2], 0.0,
                        rsm[:, :, 0:NG2], OP.max, OP.add)
                    nc.scalar.activation(s2[:, :, 0:NG2], s2[:, :, 0:NG2],
                                         AF.Sigmoid)

                    mv = m32[:].rearrange("p (m c) -> p m c", m=NM)
                    tmp1 = workp.tile([128, NM, BF], f32, tag="tmp1")
                    tmp2 = workp.tile([128, NM, BF], f32, tag="tmp2")
                    # m = i2*m + (1-i2)*g2 ; h = m*o2
                    nc.vector.tensor_tensor(tmp1[:], s2[:, :, 0:BF], mv, OP.mult)
                    nc.vector.tensor_tensor(tmp2[:], s2[:, :, 0:BF],
                                            s2[:, :, BF:2 * BF], OP.mult)
                    nc.vector.tensor_tensor(tmp2[:], s2[:, :, BF:2 * BF],
                                            tmp2[:], OP.subtract)
                    nc.vector.tensor_tensor(mv, tmp1[:], tmp2[:], OP.add)
                    hnew32 = workp.tile([128, NM * BF], f32, tag="hnew32")
                    nc.vector.tensor_tensor(
                        hnew32[:].rearrange("p (m c) -> p m c", m=NM),
                        mv, s2[:, :, 2 * BF:3 * BF], OP.mult)
                    hmn16 = workp.tile([128, NM, 2 * BF], f16, tag="hmn16")
                    nc.vector.tensor_copy(
                        hmn16[:, :, 0:BF],
                        hnew32[:].rearrange("p (m c) -> p m c", m=NM))
                    nc.vector.tensor_copy(
                        hmn16[:, :, BF:2 * BF],
                        m32[:].rearrange("p (m c) -> p m c", m=NM))

                    # ---------- all-gather [h|m] + window work ----------
                    if t < T - 1:
                        agB_in = dramp.tile([RPC, 2 * BF], f16, tag="agB_in")
                        nc.sync.dma_start(
                            agB_in.rearrange("(m p) c -> p m c", p=128),
                            hmn16[:])
                        agB_out = dramp.tile([N, 2 * BF], f16, tag="agB_out",
                                             addr_space="Shared")
                        nc.gpsimd.collective_compute(
                            "AllGather", OP.bypass,
                            replica_groups=[list(range(NCORES))],
                            ins=[agB_in[:]], outs=[agB_out[:]])

                    nc.scalar.dma_start(
                        hs[t].rearrange("m b p f -> p m b f"),
                        hnew32[:].rearrange("p (m b f) -> p m b f", m=NM, b=B))
                    if t < T - 1:
                        rgx_cur = rgx_next
                        for q in range(4):
                            nc.sync.dma_start(
                                hm16[:, q * NK // 4:(q + 1) * NK // 4, :],
                                agB_out.rearrange("(k p) c -> p k c", p=128)
                                [:, q * NK // 4:(q + 1) * NK // 4, :])

                nc.scalar.dma_start(
                    lc[:].rearrange("m b p f -> p m b f"),
                    c32[:].rearrange("p (m b f) -> p m b f", m=NM, b=B))
                nc.scalar.dma_start(
                    lm[:].rearrange("m b p f -> p m b f"),
                    m32[:].rearrange("p (m b f) -> p m b f", m=NM, b=B))

    _legalize_waits(nc)
    return nc


def _legalize_waits(nc):
    """Walrus accepts at most 1 sync-wait per instruction (2 for
    EventSemaphore). Move excess waits onto standalone EventSemaphore
    instructions on the same engine, inserted just before."""
    import concourse.mybir as mybir

    n_split = 0
    for fn in nc.m.functions:
        for bb in fn.blocks:
            newl = []
            changed = False
            for ins in bb.instructions:
                si = ins.sync_info
                waits = list(si.on_wait) if (si is not None and si.on_wait) else []
                cap = 2 if isinstance(ins, mybir.InstEventSemaphore) else 1
                if len(waits) > cap:
                    extra, keep = waits[:-cap], waits[-cap:]
                    for i in range(0, len(extra), 2):
                        ev = mybir.InstEventSemaphore(
                            name=f"{ins.name}_xw{i}",
                            engine=ins.engine,
                            sync_info=mybir.SyncInfo(
                                on_wait=list(extra[i:i + 2]), on_update=[]),
                        )
                        newl.append(ev)
                        n_split += 1
                    ins.sync_info = mybir.SyncInfo(
                        on_wait=list(keep), on_update=list(si.on_update))
                    changed = True
                newl.append(ins)
            if changed:
                bb.instructions = newl
    return n_split


def run(inputs, trace=False):
    from concourse.bass_utils import run_bass_kernel_spmd

    if "nc" not in _cache:
        _cache["nc"] = _build_nc()
    nc = _cache["nc"]

    x = np.ascontiguousarray(inputs["x"], dtype=np.float32)
    adj = np.ascontiguousarray(inputs["adj"], dtype=np.float32)
    # x rearranged to [t, n, (b, f)]
    xr = np.ascontiguousarray(x.transpose(1, 2, 0, 3)).reshape(T, N, B * F)
    ident = np.eye(128, dtype=np.float32)

    def wbd(w, bvec):
        # block-diagonal over batch, columns gate-major, bias as last row:
        # out[b*F+fi, g*B*F + b*F + fo] = w[fi, g*F+fo]; out[64, col] = bias
        w = np.asarray(w, np.float32)
        f, gtot = w.shape
        ng = gtot // F
        out = np.zeros((B * f + 1, ng * B * F), np.float32)
        for b in range(B):
            for g in range(ng):
                out[b * f:(b + 1) * f,
                    g * B * F + b * F:g * B * F + (b + 1) * F] = \
                    w[:, g * F:(g + 1) * F]
        arr = np.asarray(bvec, np.float32).reshape(ng, F)
        out[B * f, :] = np.broadcast_to(arr[:, None, :], (ng, B, F)).reshape(-1)
        return out

    common = {
        "xr": xr,
        "wx": wbd(inputs["Wx"], inputs["bx"]),
        "wh": wbd(inputs["Wh"], inputs["bh"]),
        "wsh": wbd(inputs["Wsh"], inputs["bsh"]),
        "wsm": wbd(inputs["Wsm"], inputs["bsm"]),
        "ident": ident,
    }
    in_maps = []
    for c in range(NCORES):
        m = dict(common)
        m["adjT"] = np.ascontiguousarray(adj[c * RPC:(c + 1) * RPC, :].T)
        in_maps.append(m)

    res = run_bass_kernel_spmd(
        nc, in_maps, core_ids=list(range(NCORES)), trace=trace)

    hs_parts, lc_parts, lm_parts = [], [], []
    for c in range(NCORES):
        r = res.results[c]
        # hs [T, NM, B, 128, F] -> [B, T, RPC, F]
        hs_parts.append(r["hs"].transpose(2, 0, 1, 3, 4).reshape(B, T, RPC, F))
        lc_parts.append(r["lc"].transpose(1, 0, 2, 3).reshape(B, RPC, F))
        lm_parts.append(r["lm"].transpose(1, 0, 2, 3).reshape(B, RPC, F))
    hidden = np.concatenate(hs_parts, axis=2)
    last_c = np.concatenate(lc_parts, axis=1)
    last_m = np.concatenate(lm_parts, axis=1)
    last_h = np.ascontiguousarray(hidden[:, T - 1])
    return (hidden, last_h, last_c, last_m), res


def kernel(**inputs):
    out, _ = run(inputs, trace=False)
    return out


# revision 27
# speedup vs baseline: 1.0298x; 1.0298x over previous
"""Trainium2 Bass kernel for nn_Encoder_80616536146562 (graph-LSTM encoder).

Reference computation (B=4, T=12, N=4096, F=16):
  per step t:
    gx = relu(adj @ (x_t @ Wx) + bx); gh = relu(adj @ (h @ Wh) + bh)
    LSTM gates -> c, h2
    sh = relu(adj @ (h2 @ Wsh) + bsh); sm = relu(adj @ (m @ Wsm) + bsm)
    memory gates -> m, h = m * o2
  outputs: hidden_states [B,T,N,F], last_h, last_c, last_m

Strategy: 8-way row-shard of adj (512 rows/core) kept resident in SBUF as
fp16 (scaled by 4096 to avoid fp16 subnormals; un-scaled on PSUM eviction).
Associativity: compute Y = adj @ z first (F cols per state), then tiny
Y @ W_blockdiag matmuls (batch folded block-diagonally, gate-major columns,
bias folded in via an appended ones-row on lhsT). Node states h/m/h2 are
all-gathered in fp16 across the 8 cores twice per step via collective_compute
through DRAM bounce buffers. The x-side conv for step t+1 (adj @ x_{t+1} and
its gate pre-activation) is computed inside step t's collective windows.
"""
import sys

if "/opt/trn_rl_repo" not in sys.path:
    sys.path.insert(0, "/opt/trn_rl_repo")

import numpy as np

B, T, N, F = 4, 12, 4096, 16
NCORES = 8
RPC = N // NCORES          # rows per core = 512
NM = RPC // 128            # m-tiles per core = 4
NK = N // 128              # k-tiles = 32
G1 = 4 * F                 # 64  (f|i|c|o gates)
G2 = 3 * F                 # 48  (i|g|o gates)
BF = B * F                 # 64
PBG1 = B * G1              # 256 per-m gate block
NG2 = B * G2               # 192
SCALE = 4096.0

_cache = {}


def _build_nc():
    import bass_rust as _bass_rust
    import concourse.bass as bass
    import concourse.mybir as mybir
    import concourse.tile as tile

    f32, f16 = mybir.dt.float32, mybir.dt.float16
    AF = mybir.ActivationFunctionType
    OP = mybir.AluOpType

    nc = bass.Bass(trn_type="TRN2", num_devices=NCORES)

    # ---- per-core inputs ----
    adjT = nc.dram_tensor("adjT", [N, RPC], f32, kind="ExternalInput")
    xr = nc.dram_tensor("xr", [T, N, BF], f32, kind="ExternalInput")
    # block-diagonal gate-major weights with bias as final (65th) row
    wx = nc.dram_tensor("wx", [BF + 1, PBG1], f32, kind="ExternalInput")
    wh = nc.dram_tensor("wh", [BF + 1, PBG1], f32, kind="ExternalInput")
    wsh = nc.dram_tensor("wsh", [BF + 1, NG2], f32, kind="ExternalInput")
    wsm = nc.dram_tensor("wsm", [BF + 1, NG2], f32, kind="ExternalInput")
    ident = nc.dram_tensor("ident", [128, 128], f32, kind="ExternalInput")
    # ---- per-core outputs ----
    hs = nc.dram_tensor("hs", [T, NM, B, 128, F], f32, kind="ExternalOutput")
    lc = nc.dram_tensor("lc", [NM, B, 128, F], f32, kind="ExternalOutput")
    lm = nc.dram_tensor("lm", [NM, B, 128, F], f32, kind="ExternalOutput")

    with tile.TileContext(nc) as tc:
        with tc.tile_pool(name="const", bufs=1) as constp, \
             tc.tile_pool(name="stage", bufs=2) as stagep, \
             tc.tile_pool(name="state", bufs=1) as statep, \
             tc.tile_pool(name="dram", bufs=2, space="DRAM") as dramp, \
             tc.tile_pool(name="work", bufs=2) as workp:

            # ===== constants / weights =====
            adj16 = [constp.tile([128, RPC], f16, name=f"adj16_{k}")
                     for k in range(NK)]
            for k in range(NK):
                a32 = stagep.tile([128, RPC], f32, tag="a32", bufs=6)
                nc.sync.dma_start(a32[:], adjT[k * 128:(k + 1) * 128, :])
                nc.vector.tensor_scalar_mul(adj16[k][:], a32[:], SCALE)

            id16 = constp.tile([128, 128], f16)
            i32 = stagep.tile([128, 128], f32, tag="i32")
            nc.sync.dma_start(i32[:], ident[:])
            nc.vector.tensor_copy(id16[:], i32[:])

            w16 = {}
            for name, t_, gw in (("wx", wx, PBG1), ("wh", wh, PBG1),
                                 ("wsh", wsh, NG2), ("wsm", wsm, NG2)):
                ww = stagep.tile([BF + 1, gw], f32, tag="w32", name=f"w32_{name}")
                nc.sync.dma_start(ww[:], t_[:])
                w1 = constp.tile([BF + 1, gw], f16, name=f"w16_{name}")
                nc.vector.tensor_copy(w1[:], ww[:])
                w16[name] = w1

            # ===== recurrent state =====
            # hm16[k] cols: [h (64) | m (64)]; x16t[k]: x_{t+1}
            hm16 = statep.tile([128, NK, 2 * BF], f16)
            x16t = statep.tile([128, NK, BF], f16)
            h2g = statep.tile([128, NK * BF], f16)        # gathered h2
            c32 = statep.tile([128, NM * BF], f32)
            m32 = statep.tile([128, NM * BF], f32)
            nc.gpsimd.memset(hm16[:], 1.0)
            nc.gpsimd.memset(c32[:], 1.0)
            nc.gpsimd.memset(m32[:], 1.0)

            with tc.tile_pool(name="py1", bufs=1, space="PSUM") as py1, \
                 tc.tile_pool(name="pt1", bufs=2, space="PSUM") as pt1, \
                 tc.tile_pool(name="pg", bufs=2, space="PSUM") as pgp:

                # ---- x-phase pieces ----
                def x_load(t):
                    """DMA + cast x_t into the x tile (for next stage 1)."""
                    xt32 = workp.tile([128, NK, BF], f32, tag="xt32")
                    for h in range(2):
                        nc.scalar.dma_start(
                            xt32[:, h * NK // 2:(h + 1) * NK // 2, :],
                            xr[t].rearrange("(k p) f -> p k f", p=128)
                            [:, h * NK // 2:(h + 1) * NK // 2, :])
                    nc.vector.tensor_copy(x16t[:], xt32[:])

                def x_phase_b(ax16):
                    """Transpose Ax, append ones row, rgx = relu(Ax@Wx + bx).
                    ax16: [128, NM, BF] fp16 view of un-scaled adj@x."""
                    axt_ps = pt1.tile([BF, NM * 128], f16, tag="sm1", name="axtps")
                    for m in range(NM):
                        nc.tensor.transpose(
                            axt_ps[:, m * 128:(m + 1) * 128], ax16[:, m, :],
                            id16[:])
                    axt = workp.tile([BF + 1, NM * 128], f16, tag="axt")
                    nc.scalar.copy(axt[0:BF, :], axt_ps[:])
                    nc.gpsimd.memset(axt[BF:BF + 1, :], 1.0)
                    gxa = pgp.tile([128, NM, PBG1], f32, tag="g", name="gxa")
                    for m in range(NM):
                        nc.tensor.matmul(
                            gxa[:, m, :], axt[:, m * 128:(m + 1) * 128],
                            w16["wx"][:], start=True, stop=True)
                    rgx = workp.tile([128, NM * PBG1], f32, tag="rgx")
                    nc.scalar.activation(
                        rgx[:], gxa[:].rearrange("p m g -> p (m g)"), AF.Relu)
                    return rgx

                # prologue: Ax for t=0 via a standalone matmul pass
                x_load(0)   # x_0 into the x columns
                axp0 = pt1.tile([128, NM, BF], f32, tag="sm1", name="axp0")
                for m in range(NM):
                    for k in range(NK):
                        nc.tensor.matmul(
                            axp0[:, m, :],
                            adj16[k][:, m * 128:(m + 1) * 128],
                            x16t[:, k, :],
                            start=(k == 0), stop=(k == NK - 1))
                ax0_16 = workp.tile([128, NM, BF], f16, tag="ax16")
                nc.scalar.mul(ax0_16[:], axp0[:], 1.0 / SCALE)
                rgx_cur = x_phase_b(ax0_16)
                x_load(1)   # x_1 into the x columns for step 0's stage 1

                for t in range(T):
                    # ---- stage 1: Y1X = adj @ [h|m|x_{t+1}] ----
                    # x-part first: independent of the h|m gather, so the PE
                    # processes it during the AG_B wait
                    y1 = py1.tile([128, NM, 4 * BF], f32, tag="y1")
                    for m in range(NM):
                        for k in range(NK):
                            nc.tensor.matmul(
                                y1[:, m, 2 * BF:3 * BF],
                                adj16[k][:, m * 128:(m + 1) * 128],
                                x16t[:, k, :],
                                start=(k == 0), stop=(k == NK - 1))
                    for m in range(NM):
                        for k in range(NK):
                            nc.tensor.matmul(
                                y1[:, m, 0:2 * BF],
                                adj16[k][:, m * 128:(m + 1) * 128],
                                hm16[:, k, :],
                                start=(k == 0), stop=(k == NK - 1))
                    y1s = workp.tile([128, NM, 3 * BF], f16, tag="y1s")
                    nc.scalar.mul(y1s[:], y1[:, :, 0:3 * BF], 1.0 / SCALE)
                    t1h_ps = pt1.tile([BF, NM * 128], f16, tag="sm1")
                    t1m_ps = pt1.tile([BF, NM * 128], f16, tag="sm1", name="t1mps")
                    for m in range(NM):
                        nc.tensor.transpose(
                            t1h_ps[:, m * 128:(m + 1) * 128],
                            y1s[:, m, 0:BF], id16[:])
                        nc.tensor.transpose(
                            t1m_ps[:, m * 128:(m + 1) * 128],
                            y1s[:, m, BF:2 * BF], id16[:])
                    t1h = workp.tile([BF + 1, NM * 128], f16, tag="t1h")
                    nc.scalar.copy(t1h[0:BF, :], t1h_ps[:])
                    nc.gpsimd.memset(t1h[BF:BF + 1, :], 1.0)
                    t1m = workp.tile([BF + 1, NM * 128], f16, tag="t1m")
                    nc.vector.tensor_copy(t1m[0:BF, :], t1m_ps[:])
                    nc.gpsimd.memset(t1m[BF:BF + 1, :], 1.0)

                    gh = pgp.tile([128, NM, PBG1], f32, tag="g")
                    for m in range(NM):
                        nc.tensor.matmul(
                            gh[:, m, :], t1h[:, m * 128:(m + 1) * 128],
                            w16["wh"][:], start=True, stop=True)
                    # s = rgx + relu(gh + bh)   (bias already inside gh)
                    s = workp.tile([128, NM, PBG1], f32, tag="s", bufs=1)
                    nc.vector.scalar_tensor_tensor(
                        s[:].rearrange("p m g -> p (m g)"),
                        gh[:].rearrange("p m g -> p (m g)"), 0.0,
                        rgx_cur[:], OP.max, OP.add)
                    # gate-major: per m [f | i | c | o], (b,f) inner
                    nc.scalar.activation(s[:, :, 0:2 * BF], s[:, :, 0:2 * BF],
                                         AF.Sigmoid)
                    nc.scalar.activation(s[:, :, 2 * BF:3 * BF],
                                         s[:, :, 2 * BF:3 * BF], AF.Tanh)
                    nc.scalar.activation(s[:, :, 3 * BF:4 * BF],
                                         s[:, :, 3 * BF:4 * BF], AF.Sigmoid)
                    cv = c32[:].rearrange("p (m c) -> p m c", m=NM)
                    tmp1 = workp.tile([128, NM, BF], f32, tag="tmp1")
                    tmp2 = workp.tile([128, NM, BF], f32, tag="tmp2")
                    nc.vector.tensor_tensor(tmp1[:], s[:, :, 0:BF], cv, OP.mult)
                    nc.vector.tensor_tensor(tmp2[:], s[:, :, BF:2 * BF],
                                            s[:, :, 2 * BF:3 * BF], OP.mult)
                    nc.vector.tensor_tensor(cv, tmp1[:], tmp2[:], OP.add)
                    th = workp.tile([128, NM * BF], f32, tag="th")
                    nc.scalar.activation(th[:], c32[:], AF.Tanh)
                    h2_16 = workp.tile([128, NM, BF], f16, tag="h2_16")
                    nc.vector.tensor_tensor(
                        h2_16[:], s[:, :, 3 * BF:4 * BF],
                        th[:].rearrange("p (m c) -> p m c", m=NM), OP.mult)

                    # ---------- all-gather h2 ----------
                    agA_in = dramp.tile([RPC, BF], f16, tag="agA_in")
                    nc.sync.dma_start(
                        agA_in.rearrange("(m p) f -> p m f", p=128), h2_16[:])
                    agA_out = dramp.tile([N, BF], f16, tag="agA_out",
                                         addr_space="Shared")
                    nc.gpsimd.collective_compute(
                        "AllGather", OP.bypass,
                        replica_groups=[list(range(NCORES))],
                        ins=[agA_in[:]], outs=[agA_out[:]])

                    # window work: sm matmul (needs only t1m) + x conv of t+1
                    sm = pgp.tile([128, NM, PBG1], f32, tag="g", name="sm")
                    for m in range(NM):
                        nc.tensor.matmul(
                            sm[:, m, 0:NG2], t1m[:, m * 128:(m + 1) * 128],
                            w16["wsm"][:], start=True, stop=True)
                    rsm = workp.tile([128, NM, PBG1], f32, tag="rsm", bufs=1)
                    nc.scalar.activation(
                        rsm[:, :, 0:NG2], sm[:, :, 0:NG2], AF.Relu)
                    if t < T - 1:
                        rgx_next = x_phase_b(y1s[:, :, 2 * BF:3 * BF])

                    for q in range(4):
                        nc.sync.dma_start(
                            h2g[:, q * NK // 4 * BF:(q + 1) * NK // 4 * BF]
                            .rearrange("p (k f) -> p k f", k=NK // 4),
                            agA_out.rearrange("(k p) f -> p k f", p=128)
                            [:, q * NK // 4:(q + 1) * NK // 4, :])

                    # ---------- stage 2: Y2 = adj @ h2 ----------
                    y2 = pt1.tile([128, NM, BF], f32, tag="sm1", name="y2")
                    for m in range(NM):
                        for k in range(NK):
                            nc.tensor.matmul(
                                y2[:, m, :],
                                adj16[k][:, m * 128:(m + 1) * 128],
                                h2g[:, k * BF:(k + 1) * BF],
                                start=(k == 0), stop=(k == NK - 1))
                    y2s = workp.tile([128, NM, BF], f16, tag="y2s")
                    nc.scalar.mul(y2s[:], y2[:], 1.0 / SCALE)
                    t2_ps = pt1.tile([BF, NM * 128], f16, tag="sm1", name="t2ps")
                    for m in range(NM):
                        nc.tensor.transpose(
                            t2_ps[:, m * 128:(m + 1) * 128], y2s[:, m, :], id16[:])
                    t2 = workp.tile([BF + 1, NM * 128], f16, tag="t2")
                    nc.scalar.copy(t2[0:BF, :], t2_ps[:])
                    nc.gpsimd.memset(t2[BF:BF + 1, :], 1.0)

                    sh = pgp.tile([128, NM, PBG1], f32, tag="g", name="sh")
                    for m in range(NM):
                        nc.tensor.matmul(
                            sh[:, m, 0:NG2], t2[:, m * 128:(m + 1) * 128],
                            w16["wsh"][:], start=True, stop=True)
                    # s2 = sigmoid(relu(sh+bsh) + relu(sm+bsm)); gate-major i|g|o
                    s2 = workp.tile([128, NM, PBG1], f32, tag="s2", bufs=1)
                    nc.vector.scalar_tensor_tensor(
                        s2[:, :, 0:NG2], sh[:, :, 0:NG2], 0.0,
                        rsm[:, :, 0:NG2], OP.max, OP.add)
                    nc.scalar.activation(s2[:, :, 0:NG2], s2[:, :, 0:NG2],
                                         AF.Sigmoid)

                    mv = m32[:].rearrange("p (m c) -> p m c", m=NM)
                    tmp1 = workp.tile([128, NM, BF], f32, tag="tmp1")
                    tmp2 = workp.tile([128, NM, BF], f32, tag="tmp2")
                    # m = i2*m + (1-i2)*g2 ; h = m*o2
                    nc.vector.tensor_tensor(tmp1[:], s2[:, :, 0:BF], mv, OP.mult)
                    nc.vector.tensor_tensor(tmp2[:], s2[:, :, 0:BF],
                                            s2[:, :, BF:2 * BF], OP.mult)
                    nc.vector.tensor_tensor(tmp2[:], s2[:, :, BF:2 * BF],
                                            tmp2[:], OP.subtract)
                    nc.vector.tensor_tensor(mv, tmp1[:], tmp2[:], OP.add)
                    hnew32 = workp.tile([128, NM * BF], f32, tag="hnew32")
                    nc.vector.tensor_tensor(
                        hnew32[:].rearrange("p (m c) -> p m c", m=NM),
                        mv, s2[:, :, 2 * BF:3 * BF], OP.mult)
                    hmn16 = workp.tile([128, NM, 2 * BF], f16, tag="hmn16")
                    nc.vector.tensor_copy(
                        hmn16[:, :, 0:BF],
                        hnew32[:].rearrange("p (m c) -> p m c", m=NM))
                    nc.vector.tensor_copy(
                        hmn16[:, :, BF:2 * BF],
                        m32[:].rearrange("p (m c) -> p m c", m=NM))

                    # ---------- all-gather [h|m] + window work ----------
                    if t < T - 1:
                        agB_in = dramp.tile([RPC, 2 * BF], f16, tag="agB_in")
                        nc.sync.dma_start(
                            agB_in.rearrange("(m p) c -> p m c", p=128),
                            hmn16[:])
                        agB_out = dramp.tile([N, 2 * BF], f16, tag="agB_out",
                                             addr_space="Shared")
                        nc.gpsimd.collective_compute(
                            "AllGather", OP.bypass,
                            replica_groups=[list(range(NCORES))],
                            ins=[agB_in[:]], outs=[agB_out[:]])

                    nc.scalar.dma_start(
                        hs[t].rearrange("m b p f -> p m b f"),
                        hnew32[:].rearrange("p (m b f) -> p m b f", m=NM, b=B))
                    if t < T - 1:
                        rgx_cur = rgx_next
                        if t + 2 < T:
                            x_load(t + 2)
                        for q in range(4):
                            nc.sync.dma_start(
                                hm16[:, q * NK // 4:(q + 1) * NK // 4, :],
                                agB_out.rearrange("(k p) c -> p k c", p=128)
                                [:, q * NK // 4:(q + 1) * NK // 4, :])

                nc.scalar.dma_start(
                    lc[:].rearrange("m b p f -> p m b f"),
                    c32[:].rearrange("p (m b f) -> p m b f", m=NM, b=B))
                nc.scalar.dma_start(
                    lm[:].rearrange("m b p f -> p m b f"),
                    m32[:].rearrange("p (m b f) -> p m b f", m=NM, b=B))

    _legalize_waits(nc)
    return nc


def _legalize_waits(nc):
    """Walrus accepts at most 1 sync-wait per instruction (2 for
    EventSemaphore). Move excess waits onto standalone EventSemaphore
    instructions on the same engine, inserted just before."""
    import concourse.mybir as mybir

    n_split = 0
    for fn in nc.m.functions:
        for bb in fn.blocks:
            newl = []
            changed = False
            for ins in bb.instructions:
                si = ins.sync_info
                waits = list(si.on_wait) if (si is not None and si.on_wait) else []
                cap = 2 if isinstance(ins, mybir.InstEventSemaphore) else 1
                if len(waits) > cap:
                    extra, keep = waits[:-cap], waits[-cap:]
                    for i in range(0, len(extra), 2):
                        ev = mybir.InstEventSemaphore(
                            name=f"{ins.name}_xw{i}",
                            engine=ins.engine,
                            sync_info=mybir.SyncInfo(
                                on_wait=list(extra[i:i + 2]), on_update=[]),
                        )
                        newl.append(ev)
                        n_split += 1
                    ins.sync_info = mybir.SyncInfo(
                        on_wait=list(keep), on_update=list(si.on_update))
                    changed = True
                newl.append(ins)
            if changed:
                bb.instructions = newl
    return n_split


def run(inputs, trace=False):
    from concourse.bass_utils import run_bass_kernel_spmd

    if "nc" not in _cache:
        _cache["nc"] = _build_nc()
    nc = _cache["nc"]

    x = np.ascontiguousarray(inputs["x"], dtype=np.float32)
    adj = np.ascontiguousarray(inputs["adj"], dtype=np.float32)
    # x rearranged to [t, n, (b, f)]
    xr = np.ascontiguousarray(x.transpose(1, 2, 0, 3)).reshape(T, N, B * F)
    ident = np.eye(128, dtype=np.float32)

    def wbd(w, bvec):
        # block-diagonal over batch, columns gate-major, bias as last row:
        # out[b*F+fi, g*B*F + b*F + fo] = w[fi, g*F+fo]; out[64, col] = bias
        w = np.asarray(w, np.float32)
        f, gtot = w.shape
        ng = gtot // F
        out = np.zeros((B * f + 1, ng * B * F), np.float32)
        for b in range(B):
            for g in range(ng):
                out[b * f:(b + 1) * f,
                    g * B * F + b * F:g * B * F + (b + 1) * F] = \
                    w[:, g * F:(g + 1) * F]
        arr = np.asarray(bvec, np.float32).reshape(ng, F)
        out[B * f, :] = np.broadcast_to(arr[:, None, :], (ng, B, F)).reshape(-1)
        return out

    common = {
        "xr": xr,
        "wx": wbd(inputs["Wx"], inputs["bx"]),
        "wh": wbd(inputs["Wh"], inputs["bh"]),
        "wsh": wbd(inputs["Wsh"], inputs["bsh"]),
        "wsm": wbd(inputs["Wsm"], inputs["bsm"]),
        "ident": ident,
    }
    in_maps = []
    for c in range(NCORES):
        m = dict(common)
        m["adjT"] = np.ascontiguousarray(adj[c * RPC:(c + 1) * RPC, :].T)
        in_maps.append(m)

    res = run_bass_kernel_spmd(
        nc, in_maps, core_ids=list(range(NCORES)), trace=trace)

    hs_parts, lc_parts, lm_parts = [], [], []
    for c in range(NCORES):
        r = res.results[c]
        # hs [T, NM, B, 128, F] -> [B, T, RPC, F]
        hs_parts.append(r["hs"].transpose(2, 0, 1, 3, 4).reshape(B, T, RPC, F))
        lc_parts.append(r["lc"].transpose(1, 0, 2, 3).reshape(B, RPC, F))
        lm_parts.append(r["lm"].transpose(1, 0, 2, 3).reshape(B, RPC, F))
    hidden = np.concatenate(hs_parts, axis=2)
    last_c = np.concatenate(lc_parts, axis=1)
    last_m = np.concatenate(lm_parts, axis=1)
    last_h = np.ascontiguousarray(hidden[:, T - 1])
    return (hidden, last_h, last_c, last_m), res


def kernel(**inputs):
    out, _ = run(inputs, trace=False)
    return out
